# revision 21
# baseline (speedup 1.0000x reference)
"""MQA cross-attention kernel for Trainium2 (8 NeuronCores, SPMD).

Problem: out = softmax(LN(x)@Wq reshaped to 12 heads @ SCALE @ (ctx@Wk)^T
                        + mask) @ (ctx@Wv) @ Wo
Shapes: x [4,2048,768], context [4,2048,768], mask [4,2048,2048] (zeros),
        Wq [768,768], Wkv [768,128] (MQA: single shared K/V head of 64),
        Wo [768,768].

Sharding: 8 cores = (batch 0..3) x (query-half 0..1). Each core runs all 12
heads on 1024 queries x 2048 keys of one batch. Outputs are disjoint slices
-> host-side concat, no cross-core reduction.

Per-core layout ("scheme T"): sim is computed TRANSPOSED, simT[j, (h,i)], so
the exp output attnT already has the contraction axis j on partitions for the
attn@V matmul, and the softmax denominator falls out of an appended
ones-column in the V matmul (no full-matrix transposes, no reductions over
the big axis). All matmuls run as float32r (tf32-class, ~1.5e-4 rel err).
"""
import sys

import numpy as np

for _p in ('/opt/trn_rl_repo', '/root/.axon_site/_ro/trn_rl_repo'):
    if _p not in sys.path:
        sys.path.append(_p)

HEADS = 12
DH = 64
SCALE = DH ** -0.5
B, N, J, D = 4, 2048, 2048, 768
NCORES = 8
NI = N // 2          # queries per core
KD = D // 128        # 6 contraction k-tiles
NJT = J // 128       # 16 key tiles
NPAIR = HEADS // 2   # 6 head pairs
EPS = 1e-5

_CACHE = {}


def _build_nc(debug_taps=False):
    import concourse.bass as bass
    from concourse import bacc, tile
    import concourse.mybir as mybir
    from concourse.masks import make_identity

    f32 = mybir.dt.float32
    f32r = mybir.dt.float32r
    AF = mybir.ActivationFunctionType
    Alu = mybir.AluOpType

    nc = bacc.Bacc('TRN2', target_bir_lowering=False, debug=False)

    dbg = {}
    if debug_taps:
        for name, shape in [('dbg_kvT', [128, J]), ('dbg_qT', [128, KD, NI]),
                            ('dbg_attnT', [128, 1024]), ('dbg_ave', [128, 512]),
                            ('dbg_avo', [128, 512]), ('dbg_bce', [128, 512]),
                            ('dbg_bco', [128, 512]), ('dbg_aoT', [128, KD, NI]),
                            ('dbg_v1e', [128, NJT, 72]),
                            ('dbg_v1o', [128, NJT, 128])]:
            dbg[name] = nc.dram_tensor(name, shape, f32, kind='ExternalOutput')

    x_d = nc.dram_tensor('x', [NI, D], f32, kind='ExternalInput')
    ctx_d = nc.dram_tensor('ctx', [J, D], f32, kind='ExternalInput')
    lnw_d = nc.dram_tensor('lnw', [D], f32, kind='ExternalInput')
    wq_d = nc.dram_tensor('wq', [D, D], f32, kind='ExternalInput')
    wkv_d = nc.dram_tensor('wkv', [D, 2 * DH], f32, kind='ExternalInput')
    wo_d = nc.dram_tensor('wo', [D, D], f32, kind='ExternalInput')
    out_d = nc.dram_tensor('out', [NI, D], f32, kind='ExternalOutput')

    with tile.TileContext(nc) as tc:
        from contextlib import ExitStack
        with ExitStack() as ctx:
            singles = ctx.enter_context(tc.tile_pool(name='singles', bufs=1))
            xt_pool = ctx.enter_context(tc.tile_pool(name='xt', bufs=3))
            ln_pool = ctx.enter_context(tc.tile_pool(name='ln', bufs=4))
            actT_pool = ctx.enter_context(tc.tile_pool(name='actT', bufs=2))
            attn_pool = ctx.enter_context(tc.tile_pool(name='attn', bufs=3))
            den_pool = ctx.enter_context(tc.tile_pool(name='den', bufs=2))
            bc_pool = ctx.enter_context(tc.tile_pool(name='bc', bufs=4))
            out_pool = ctx.enter_context(tc.tile_pool(name='outp', bufs=2))
            ps_big = ctx.enter_context(
                tc.tile_pool(name='ps_big', bufs=2, space='PSUM'))
            ps_small = ctx.enter_context(
                tc.tile_pool(name='ps_small', bufs=4, space='PSUM'))
            dram_pool = ctx.enter_context(
                tc.tile_pool(name='dram', bufs=4, space='DRAM'))

            # ---- constants / weights ----
            ident = singles.tile([128, 128], f32)
            make_identity(nc, ident)
            eps_t = singles.tile([128, 1], f32)
            nc.vector.memset(eps_t, EPS)

            lnw_bc = singles.tile([128, D], f32)
            nc.sync.dma_start(out=lnw_bc,
                              in_=lnw_d[None, :].to_broadcast((128, D)))

            wq_sb = singles.tile([128, KD, D], f32r)
            nc.sync.dma_start(
                out=wq_sb,
                in_=wq_d[:, :].rearrange('(ko p) f -> p ko f', p=128).bitcast(f32r))
            wkv_sb = singles.tile([128, KD, 2 * DH], f32r)
            nc.sync.dma_start(
                out=wkv_sb,
                in_=wkv_d[:, :].rearrange('(ko p) f -> p ko f', p=128).bitcast(f32r))
            wo_sb = singles.tile([128, KD, D], f32r)
            nc.sync.dma_start(
                out=wo_sb,
                in_=wo_d[:, :].rearrange('(ko p) f -> p ko f', p=128).bitcast(f32r))

            # persistent big tensors
            kvT_sb = singles.tile([128, J], f32r)        # rows 0-63 kT, 64-127 vT
            kthi_sb = singles.tile([128, J], f32r)       # rows 64-127: copy of kT
            v1e = singles.tile([128, NJT, 72], f32r)     # [v | 1] per j-tile
            v1o = singles.tile([128, NJT, 128], f32r)    # [1 | pad63 | v]
            qT_sb = singles.tile([128, KD, NI], f32r)
            aoT_sb = singles.tile([128, KD, NI], f32r)

            # memset can't write f32r; initialize via copies from f32 scratch.
            # v1e cols 65-71 are never read, so only its ones-column needs init.
            zeros_t = singles.tile([128, 1024], f32)
            nc.vector.memset(zeros_t, 0.0)
            ones_t = singles.tile([128, 1], f32)
            nc.vector.memset(ones_t, 1.0)
            v1o_flat = v1o.rearrange('p a b -> p (a b)')
            for z0 in range(0, NJT * 128, 1024):
                nc.vector.tensor_copy(v1o_flat[:, z0:z0 + 1024], zeros_t)
            nc.vector.tensor_copy(v1e[:, :, 64:65],
                                  ones_t[:, None, :].to_broadcast((128, NJT, 1)))
            nc.vector.tensor_copy(v1o[:, :, 0:1],
                                  ones_t[:, None, :].to_broadcast((128, NJT, 1)))

            # ---- phase A: ctx -> ctxT chunks -> kvT ----
            for jc in range(4):           # 512-wide chunks of j
                ctxT_c = actT_pool.tile([128, KD, 512], f32r, tag='actT')
                for t in range(4):        # four 128-row ctx tiles per chunk
                    jt = jc * 4 + t
                    c_t = xt_pool.tile([128, D], f32, tag='xt')
                    nc.sync.dma_start(out=c_t, in_=ctx_d[jt * 128:(jt + 1) * 128, :])
                    for g in range(2):    # transpose 6 blocks in groups of 3
                        tp = ps_small.tile([128, 512], f32, tag='small')
                        for b_ in range(3):
                            kd = g * 3 + b_
                            nc.tensor.transpose(
                                tp[:, b_ * 128:(b_ + 1) * 128],
                                c_t[:, kd * 128:(kd + 1) * 128], ident)
                        nc.vector.tensor_copy(
                            ctxT_c[:, g * 3:(g + 1) * 3, t * 128:(t + 1) * 128],
                            tp[:, 0:384].rearrange('p (b i) -> p b i', b=3))
                kv_ps = ps_small.tile([128, 512], f32, tag='small')
                for k in range(KD):
                    nc.tensor.matmul(kv_ps[:, :], wkv_sb[:, k, :].bitcast(f32r),
                                     ctxT_c[:, k, :], start=(k == 0), stop=(k == KD - 1))
                # kv_ps rows: 0-63 = kT chunk, 64-127 = vT chunk -- wait, no:
                # kv_ps is [c=128, j=512]: c = kv feature (0-63 k, 64-127 v)
                nc.vector.tensor_copy(kvT_sb[:, jc * 512:(jc + 1) * 512], kv_ps)

            # vT -> v (j on partitions) for the AV matmul lhsT
            for jt in range(NJT):
                vp = ps_small.tile([128, 512], f32, tag='small')
                nc.tensor.transpose(
                    vp[:, 0:64], kvT_sb[64:128, jt * 128:(jt + 1) * 128].bitcast(f32),
                    ident[64:128, 64:128])
                nc.vector.tensor_copy(v1e[:, jt, 0:64], vp[:, 0:64])
                nc.vector.tensor_copy(v1o[:, jt, 64:128], vp[:, 0:64])
            # second copy of kT at partitions 64-127 (for row-tiled sim)
            nc.sync.dma_start(out=kthi_sb[64:128, :], in_=kvT_sb[0:64, :])
            if debug_taps:
                nc.sync.dma_start(out=dbg['dbg_kvT'][:, :],
                                  in_=kvT_sb.bitcast(f32))
                nc.sync.dma_start(out=dbg['dbg_v1e'][:, :, :],
                                  in_=v1e.bitcast(f32))
                nc.sync.dma_start(out=dbg['dbg_v1o'][:, :, :],
                                  in_=v1o.bitcast(f32))

            # ---- phase B: LN(x) -> xnT chunks -> qT ----
            for ic in range(2):           # 512-wide chunks of i
                xnT_c = actT_pool.tile([128, KD, 512], f32r, tag='actT')
                for t in range(4):
                    it = ic * 4 + t
                    x_t = xt_pool.tile([128, D], f32, tag='xt')
                    nc.sync.dma_start(out=x_t, in_=x_d[it * 128:(it + 1) * 128, :])
                    stats = ln_pool.tile([128, 3, 6], f32, tag='stats')
                    for s in range(3):
                        nc.vector.bn_stats(out=stats[:, s, :],
                                           in_=x_t[:, s * 256:(s + 1) * 256])
                    mv = ln_pool.tile([128, 2], f32, tag='mv')
                    nc.vector.bn_aggr(out=mv, in_=stats)
                    # rstd = exp(-0.5 * ln(var + eps)); Ln+Exp share one table set
                    lnv = ln_pool.tile([128, 1], f32, tag='lnv')
                    nc.scalar.activation(out=lnv, in_=mv[:, 1:2], func=AF.Ln,
                                         bias=eps_t)
                    rstd = ln_pool.tile([128, 1], f32, tag='rstd')
                    nc.scalar.activation(out=rstd, in_=lnv, func=AF.Exp, scale=-0.5)
                    xn_t = xt_pool.tile([128, D], f32, tag='xn')
                    nc.vector.tensor_scalar(out=xn_t, in0=x_t,
                                            scalar1=mv[:, 0:1], scalar2=rstd,
                                            op0=Alu.subtract, op1=Alu.mult)
                    nc.vector.tensor_mul(out=xn_t, in0=xn_t, in1=lnw_bc)
                    for g in range(2):
                        tp = ps_small.tile([128, 512], f32, tag='small')
                        for b_ in range(3):
                            kd = g * 3 + b_
                            nc.tensor.transpose(
                                tp[:, b_ * 128:(b_ + 1) * 128],
                                xn_t[:, kd * 128:(kd + 1) * 128], ident)
                        nc.vector.tensor_copy(
                            xnT_c[:, g * 3:(g + 1) * 3, t * 128:(t + 1) * 128],
                            tp[:, 0:384].rearrange('p (b i) -> p b i', b=3))
                for fc in range(KD):      # qT = Wq.T @ xnT
                    q_ps = ps_small.tile([128, 512], f32, tag='small')
                    for k in range(KD):
                        nc.tensor.matmul(
                            q_ps, wq_sb[:, k, fc * 128:(fc + 1) * 128].bitcast(f32r),
                            xnT_c[:, k, :], start=(k == 0), stop=(k == KD - 1))
                    nc.vector.tensor_copy(
                        qT_sb[:, fc, ic * 512:(ic + 1) * 512], q_ps)

            if debug_taps:
                nc.sync.dma_start(out=dbg['dbg_qT'][:, :, :],
                                  in_=qT_sb.bitcast(f32))

            # ---- phase C: attention ----
            for pr in range(NPAIR):
                for ih in range(2):
                    i0 = ih * 512
                    av_e = ps_small.tile([128, 512], f32, tag='small')
                    av_o = ps_small.tile([128, 512], f32, tag='small')
                    for jt in range(NJT):
                        s_ps = ps_big.tile([128, 1024], f32, tag='big')
                        nc.tensor.matmul(
                            s_ps[:, 0:512],
                            kvT_sb[0:64, jt * 128:(jt + 1) * 128],
                            qT_sb[0:64, pr, i0:i0 + 512],
                            start=True, stop=True, tile_position=(0, 0))
                        nc.tensor.matmul(
                            s_ps[:, 512:1024],
                            kthi_sb[64:128, jt * 128:(jt + 1) * 128],
                            qT_sb[64:128, pr, i0:i0 + 512],
                            start=True, stop=True, tile_position=(64, 0))
                        attnT = attn_pool.tile([128, 1024], f32r, tag='attn')
                        nc.scalar.activation(out=attnT, in_=s_ps, func=AF.Exp)
                        if debug_taps and pr == 0 and ih == 0 and jt == 0:
                            nc.sync.dma_start(out=dbg['dbg_attnT'][:, :],
                                              in_=attnT.bitcast(f32))
                        nc.tensor.matmul(av_e[0:65, :], v1e[:, jt, 0:65],
                                         attnT[:, 0:512],
                                         start=(jt == 0), stop=(jt == NJT - 1))
                        nc.tensor.matmul(av_o[:, :], v1o[:, jt, :],
                                         attnT[:, 512:1024],
                                         start=(jt == 0), stop=(jt == NJT - 1))
                    # normalize + scatter into aoT
                    den = den_pool.tile([128, 1024], f32, tag='den')
                    nc.vector.reciprocal(out=den[64:65, 0:512], in_=av_e[64:65, :])
                    nc.vector.reciprocal(out=den[0:1, 512:1024], in_=av_o[0:1, :])
                    bce = bc_pool.tile([128, 512], f32, tag='bc')
                    bco = bc_pool.tile([128, 512], f32, tag='bc')
                    den_dr = dram_pool.tile([2, 512], f32, tag='dend')
                    nc.sync.dma_start(out=den_dr[0:1, :], in_=den[64:65, 0:512])
                    nc.sync.dma_start(out=den_dr[1:2, :], in_=den[0:1, 512:1024])
                    nc.sync.dma_start(out=bce[0:64, :],
                                      in_=den_dr[0:1, :].to_broadcast((64, 512)))
                    nc.sync.dma_start(out=bco[64:128, :],
                                      in_=den_dr[1:2, :].to_broadcast((64, 512)))
                    nc.vector.tensor_mul(
                        out=aoT_sb[0:64, pr, i0:i0 + 512],
                        in0=av_e[0:64, :], in1=bce[0:64, :])
                    nc.vector.tensor_mul(
                        out=aoT_sb[64:128, pr, i0:i0 + 512],
                        in0=av_o[64:128, :], in1=bco[64:128, :])
                    if debug_taps and pr == 0 and ih == 0:
                        av_e_sb = bc_pool.tile([128, 512], f32, tag='dbg')
                        av_o_sb = bc_pool.tile([128, 512], f32, tag='dbg')
                        nc.vector.tensor_copy(av_e_sb, av_e)
                        nc.vector.tensor_copy(av_o_sb, av_o)
                        nc.sync.dma_start(out=dbg['dbg_ave'][:, :], in_=av_e_sb)
                        nc.sync.dma_start(out=dbg['dbg_avo'][:, :], in_=av_o_sb)
                        nc.sync.dma_start(out=dbg['dbg_bce'][:, :], in_=bce)
                        nc.sync.dma_start(out=dbg['dbg_bco'][:, :], in_=bco)

            if debug_taps:
                nc.sync.dma_start(out=dbg['dbg_aoT'][:, :, :],
                                  in_=aoT_sb.bitcast(f32))

            # ---- phase D: out = aoT.T @ Wo ----
            for it in range(8):
                o_sb = out_pool.tile([128, D], f32, tag='osb')
                for mc, mw in ((0, 512), (512, 256)):
                    o_ps = ps_small.tile([128, 512], f32, tag='small')
                    for k in range(KD):
                        nc.tensor.matmul(
                            o_ps[:, 0:mw],
                            aoT_sb[:, k, it * 128:(it + 1) * 128],
                            wo_sb[:, k, mc:mc + mw],
                            start=(k == 0), stop=(k == KD - 1))
                    nc.vector.tensor_copy(o_sb[:, mc:mc + mw], o_ps[:, 0:mw])
                nc.sync.dma_start(out=out_d[it * 128:(it + 1) * 128, :], in_=o_sb)

    nc.compile()
    return nc


def _get_nc():
    if 'nc' not in _CACHE:
        _CACHE['nc'] = _build_nc()
    return _CACHE['nc']


def _numpy_fallback(x, context, mask, ln_w, Wq, Wkv, Wo):
    mu = x.mean(-1, keepdims=True)
    var = x.var(-1, keepdims=True)
    xn = (x - mu) / np.sqrt(var + EPS) * ln_w
    q = (xn @ Wq).reshape(B, N, HEADS, DH).transpose(0, 2, 1, 3) * SCALE
    kv = context @ Wkv
    k, v = kv[..., :DH], kv[..., DH:]
    sim = np.einsum('bhid,bjd->bhij', q, k) + mask[:, None, :, :]
    sim -= sim.max(-1, keepdims=True)
    a = np.exp(sim)
    a /= a.sum(-1, keepdims=True)
    out = np.einsum('bhij,bjd->bhid', a, v)
    return (out.transpose(0, 2, 1, 3).reshape(B, N, HEADS * DH) @ Wo).astype(
        np.float32)


def run_sharded(x, context, mask, ln_w, Wq, Wkv, Wo, trace=False):
    """Run the bass kernel on 8 cores; returns (out, BassKernelResults)."""
    from concourse.bass_utils import run_bass_kernel_spmd

    nc = _get_nc()
    lnw_s = np.ascontiguousarray(ln_w * SCALE, dtype=np.float32)
    in_maps = []
    for c in range(NCORES):
        b, half = divmod(c, 2)
        in_maps.append({
            'x': np.ascontiguousarray(x[b, half * NI:(half + 1) * NI, :]),
            'ctx': np.ascontiguousarray(context[b]),
            'lnw': lnw_s,
            'wq': np.ascontiguousarray(Wq),
            'wkv': np.ascontiguousarray(Wkv),
            'wo': np.ascontiguousarray(Wo),
        })
    res = run_bass_kernel_spmd(nc, in_maps, core_ids=list(range(NCORES)),
                               trace=trace)
    out = np.empty((B, N, D), dtype=np.float32)
    for c in range(NCORES):
        b, half = divmod(c, 2)
        out[b, half * NI:(half + 1) * NI, :] = res.results[c]['out']
    return out, res


def kernel(x, context, mask, ln_w, Wq, Wkv, Wo):
    x = np.asarray(x, dtype=np.float32)
    context = np.asarray(context, dtype=np.float32)
    mask = np.asarray(mask, dtype=np.float32)
    ln_w = np.asarray(ln_w, dtype=np.float32)
    Wq = np.asarray(Wq, dtype=np.float32)
    Wkv = np.asarray(Wkv, dtype=np.float32)
    Wo = np.asarray(Wo, dtype=np.float32)
    if mask.any():
        # The device kernel folds the (always-zero) additive mask away; a
        # nonzero mask is handled by the exact host fallback.
        return _numpy_fallback(x, context, mask, ln_w, Wq, Wkv, Wo)
    out, _ = run_sharded(x, context, mask, ln_w, Wq, Wkv, Wo)
    return out


# revision 31
# speedup vs baseline: 1.0432x; 1.0432x over previous
"""MQA cross-attention kernel for Trainium2 (8 NeuronCores, SPMD).

Problem: out = softmax(LN(x)@Wq reshaped to 12 heads @ SCALE @ (ctx@Wk)^T
                        + mask) @ (ctx@Wv) @ Wo
Shapes: x [4,2048,768], context [4,2048,768], mask [4,2048,2048] (zeros),
        Wq [768,768], Wkv [768,128] (MQA: single shared K/V head of 64),
        Wo [768,768].

Sharding: 8 cores = (batch 0..3) x (query-half 0..1). Each core runs all 12
heads on 1024 queries x 2048 keys of one batch. Outputs are disjoint slices
-> host-side concat, no cross-core reduction.

Per-core layout ("scheme T"): sim is computed TRANSPOSED, simT[j, (h,i)], so
the exp output attnT already has the contraction axis j on partitions for the
attn@V matmul, and the softmax denominator falls out of an appended
ones-column in the V matmul (no full-matrix transposes, no reductions over
the big axis). All matmuls run as float32r (tf32-class, ~1.5e-4 rel err).
"""
import sys

import numpy as np

for _p in ('/opt/trn_rl_repo', '/root/.axon_site/_ro/trn_rl_repo'):
    if _p not in sys.path:
        sys.path.append(_p)

HEADS = 12
DH = 64
SCALE = DH ** -0.5
B, N, J, D = 4, 2048, 2048, 768
NCORES = 8
NI = N // 2          # queries per core
KD = D // 128        # 6 contraction k-tiles
NJT = J // 128       # 16 key tiles
NPAIR = HEADS // 2   # 6 head pairs
EPS = 1e-5

_CACHE = {}


def _build_nc(debug_taps=False):
    import concourse.bass as bass
    from concourse import bacc, tile
    import concourse.mybir as mybir
    from concourse.masks import make_identity

    f32 = mybir.dt.float32
    f32r = mybir.dt.float32r
    AF = mybir.ActivationFunctionType
    Alu = mybir.AluOpType

    nc = bacc.Bacc('TRN2', target_bir_lowering=False, debug=False)

    dbg = {}
    if debug_taps:
        for name, shape in [('dbg_kvT', [128, J]), ('dbg_qT', [128, KD, NI]),
                            ('dbg_attnT', [128, 1024]), ('dbg_ave', [128, 512]),
                            ('dbg_avo', [128, 512]), ('dbg_bce', [128, 512]),
                            ('dbg_bco', [128, 512]), ('dbg_aoT', [128, KD, NI]),
                            ('dbg_v1e', [128, NJT, 72]),
                            ('dbg_v1o', [128, NJT, 128])]:
            dbg[name] = nc.dram_tensor(name, shape, f32, kind='ExternalOutput')

    x_d = nc.dram_tensor('x', [NI, D], f32, kind='ExternalInput')
    ctx_d = nc.dram_tensor('ctx', [J, D], f32, kind='ExternalInput')
    lnw_d = nc.dram_tensor('lnw', [D], f32, kind='ExternalInput')
    wq_d = nc.dram_tensor('wq', [D, D], f32, kind='ExternalInput')
    wkv_d = nc.dram_tensor('wkv', [D, 2 * DH], f32, kind='ExternalInput')
    wo_d = nc.dram_tensor('wo', [D, D], f32, kind='ExternalInput')
    out_d = nc.dram_tensor('out', [NI, D], f32, kind='ExternalOutput')

    with tile.TileContext(nc) as tc:
        from contextlib import ExitStack
        with ExitStack() as ctx:
            singles = ctx.enter_context(tc.tile_pool(name='singles', bufs=1))
            xt_pool = ctx.enter_context(tc.tile_pool(name='xt', bufs=3))
            ln_pool = ctx.enter_context(tc.tile_pool(name='ln', bufs=4))
            actT_pool = ctx.enter_context(tc.tile_pool(name='actT', bufs=2))
            attn_pool = ctx.enter_context(tc.tile_pool(name='attn', bufs=2))
            den_pool = ctx.enter_context(tc.tile_pool(name='den', bufs=2))
            bc_pool = ctx.enter_context(tc.tile_pool(name='bc', bufs=4))
            out_pool = ctx.enter_context(tc.tile_pool(name='outp', bufs=2))
            ps_big = ctx.enter_context(
                tc.tile_pool(name='ps_big', bufs=2, space='PSUM'))
            ps_small = ctx.enter_context(
                tc.tile_pool(name='ps_small', bufs=4, space='PSUM'))
            dram_pool = ctx.enter_context(
                tc.tile_pool(name='dram', bufs=4, space='DRAM'))

            # ---- constants / weights ----
            ident = singles.tile([128, 128], f32)
            make_identity(nc, ident)
            eps_t = singles.tile([128, 1], f32)
            nc.vector.memset(eps_t, EPS)

            lnw_bc = singles.tile([128, D], f32)
            nc.sync.dma_start(out=lnw_bc,
                              in_=lnw_d[None, :].to_broadcast((128, D)))

            wq_sb = singles.tile([128, KD, D], f32r)
            nc.sync.dma_start(
                out=wq_sb,
                in_=wq_d[:, :].rearrange('(ko p) f -> p ko f', p=128).bitcast(f32r))
            wkv_sb = singles.tile([128, KD, 2 * DH], f32r)
            nc.sync.dma_start(
                out=wkv_sb,
                in_=wkv_d[:, :].rearrange('(ko p) f -> p ko f', p=128).bitcast(f32r))
            wo_sb = singles.tile([128, KD, D], f32r)
            nc.sync.dma_start(
                out=wo_sb,
                in_=wo_d[:, :].rearrange('(ko p) f -> p ko f', p=128).bitcast(f32r))

            # persistent big tensors
            kvT_sb = singles.tile([128, J], f32r)        # rows 0-63 kT, 64-127 vT
            kthi_sb = singles.tile([128, J], f32r)       # rows 64-127: copy of kT
            v1e = singles.tile([128, NJT, 72], f32r)     # [v | 1] per j-tile
            v1o = singles.tile([128, NJT, 128], f32r)    # [1 | pad63 | v]
            qT_sb = singles.tile([128, KD, NI], f32r)
            aoT_sb = singles.tile([128, KD, NI], f32r)

            # memset can't write f32r; initialize via copies from f32 scratch.
            # v1e cols 65-71 are never read, so only its ones-column needs init.
            zeros_t = singles.tile([128, 1024], f32)
            nc.vector.memset(zeros_t, 0.0)
            ones_t = singles.tile([128, 1], f32)
            nc.vector.memset(ones_t, 1.0)
            v1o_flat = v1o.rearrange('p a b -> p (a b)')
            for z0 in range(0, NJT * 128, 1024):
                nc.vector.tensor_copy(v1o_flat[:, z0:z0 + 1024], zeros_t)
            nc.vector.tensor_copy(v1e[:, :, 64:65],
                                  ones_t[:, None, :].to_broadcast((128, NJT, 1)))
            nc.vector.tensor_copy(v1o[:, :, 0:1],
                                  ones_t[:, None, :].to_broadcast((128, NJT, 1)))

            # ---- phase A: ctx -> ctxT chunks -> kvT ----
            for jc in range(4):           # 512-wide chunks of j
                ctxT_c = actT_pool.tile([128, KD, 512], f32r, tag='actT')
                for t in range(4):        # four 128-row ctx tiles per chunk
                    jt = jc * 4 + t
                    c_t = xt_pool.tile([128, D], f32, tag='xt')
                    nc.sync.dma_start(out=c_t, in_=ctx_d[jt * 128:(jt + 1) * 128, :])
                    for g in range(2):    # transpose 6 blocks in groups of 3
                        tp = ps_small.tile([128, 512], f32, tag='small')
                        for b_ in range(3):
                            kd = g * 3 + b_
                            nc.tensor.transpose(
                                tp[:, b_ * 128:(b_ + 1) * 128],
                                c_t[:, kd * 128:(kd + 1) * 128], ident)
                        nc.vector.tensor_copy(
                            ctxT_c[:, g * 3:(g + 1) * 3, t * 128:(t + 1) * 128],
                            tp[:, 0:384].rearrange('p (b i) -> p b i', b=3))
                kv_ps = ps_small.tile([128, 512], f32, tag='small')
                for k in range(KD):
                    nc.tensor.matmul(kv_ps[:, :], wkv_sb[:, k, :].bitcast(f32r),
                                     ctxT_c[:, k, :], start=(k == 0), stop=(k == KD - 1))
                # kv_ps rows: 0-63 = kT chunk, 64-127 = vT chunk -- wait, no:
                # kv_ps is [c=128, j=512]: c = kv feature (0-63 k, 64-127 v)
                nc.vector.tensor_copy(kvT_sb[:, jc * 512:(jc + 1) * 512], kv_ps)

            # vT -> v (j on partitions) for the AV matmul lhsT
            for jt in range(NJT):
                vp = ps_small.tile([128, 512], f32, tag='small')
                nc.tensor.transpose(
                    vp[:, 0:64], kvT_sb[64:128, jt * 128:(jt + 1) * 128].bitcast(f32),
                    ident[64:128, 64:128])
                nc.vector.tensor_copy(v1e[:, jt, 0:64], vp[:, 0:64])
                nc.vector.tensor_copy(v1o[:, jt, 64:128], vp[:, 0:64])
            # second copy of kT at partitions 64-127 (for row-tiled sim)
            nc.sync.dma_start(out=kthi_sb[64:128, :], in_=kvT_sb[0:64, :])
            if debug_taps:
                nc.sync.dma_start(out=dbg['dbg_kvT'][:, :],
                                  in_=kvT_sb.bitcast(f32))
                nc.sync.dma_start(out=dbg['dbg_v1e'][:, :, :],
                                  in_=v1e.bitcast(f32))
                nc.sync.dma_start(out=dbg['dbg_v1o'][:, :, :],
                                  in_=v1o.bitcast(f32))

            # ---- phase B: LN(x) -> xnT chunks -> qT ----
            for ic in range(2):           # 512-wide chunks of i
                xnT_c = actT_pool.tile([128, KD, 512], f32r, tag='actT')
                for t in range(4):
                    it = ic * 4 + t
                    x_t = xt_pool.tile([128, D], f32, tag='xt')
                    nc.sync.dma_start(out=x_t, in_=x_d[it * 128:(it + 1) * 128, :])
                    stats = ln_pool.tile([128, 3, 6], f32, tag='stats')
                    for s in range(3):
                        nc.vector.bn_stats(out=stats[:, s, :],
                                           in_=x_t[:, s * 256:(s + 1) * 256])
                    mv = ln_pool.tile([128, 2], f32, tag='mv')
                    nc.vector.bn_aggr(out=mv, in_=stats)
                    # rstd = exp(-0.5 * ln(var + eps)); Ln+Exp share one table set
                    lnv = ln_pool.tile([128, 1], f32, tag='lnv')
                    nc.scalar.activation(out=lnv, in_=mv[:, 1:2], func=AF.Ln,
                                         bias=eps_t)
                    rstd = ln_pool.tile([128, 1], f32, tag='rstd')
                    nc.scalar.activation(out=rstd, in_=lnv, func=AF.Exp, scale=-0.5)
                    xn_t = xt_pool.tile([128, D], f32, tag='xn')
                    nc.vector.tensor_scalar(out=xn_t, in0=x_t,
                                            scalar1=mv[:, 0:1], scalar2=rstd,
                                            op0=Alu.subtract, op1=Alu.mult)
                    nc.vector.tensor_mul(out=xn_t, in0=xn_t, in1=lnw_bc)
                    for g in range(2):
                        tp = ps_small.tile([128, 512], f32, tag='small')
                        for b_ in range(3):
                            kd = g * 3 + b_
                            nc.tensor.transpose(
                                tp[:, b_ * 128:(b_ + 1) * 128],
                                xn_t[:, kd * 128:(kd + 1) * 128], ident)
                        nc.vector.tensor_copy(
                            xnT_c[:, g * 3:(g + 1) * 3, t * 128:(t + 1) * 128],
                            tp[:, 0:384].rearrange('p (b i) -> p b i', b=3))
                for fc in range(KD):      # qT = Wq.T @ xnT
                    q_ps = ps_small.tile([128, 512], f32, tag='small')
                    for k in range(KD):
                        nc.tensor.matmul(
                            q_ps, wq_sb[:, k, fc * 128:(fc + 1) * 128].bitcast(f32r),
                            xnT_c[:, k, :], start=(k == 0), stop=(k == KD - 1))
                    nc.vector.tensor_copy(
                        qT_sb[:, fc, ic * 512:(ic + 1) * 512], q_ps)

            if debug_taps:
                nc.sync.dma_start(out=dbg['dbg_qT'][:, :, :],
                                  in_=qT_sb.bitcast(f32))

            # ---- phase C: attention ----
            # Per (head-pair, i-half): software-pipelined over j-tiles with
            # emission order sim(k+1) -> av(k) -> exp(k+1), so the exp stream
            # on ScalarE (the critical engine) never waits: sim(k+1) runs in
            # the second PSUM buffer while exp(k) is still reading the first.
            for pr in range(NPAIR):
                for ih in range(2):
                    i0 = ih * 512
                    av_e = ps_small.tile([128, 512], f32, tag='small')
                    av_o = ps_small.tile([128, 512], f32, tag='small')

                    def sim_block(jt):
                        s_ps = ps_big.tile([128, 1024], f32, tag='big')
                        nc.tensor.matmul(
                            s_ps[:, 0:512],
                            kvT_sb[0:64, jt * 128:(jt + 1) * 128],
                            qT_sb[0:64, pr, i0:i0 + 512],
                            start=True, stop=True, tile_position=(0, 0))
                        nc.tensor.matmul(
                            s_ps[:, 512:1024],
                            kthi_sb[64:128, jt * 128:(jt + 1) * 128],
                            qT_sb[64:128, pr, i0:i0 + 512],
                            start=True, stop=True, tile_position=(64, 0))
                        return s_ps

                    def exp_block(s_ps):
                        attnT = attn_pool.tile([128, 1024], f32r, tag='attn')
                        nc.scalar.activation(out=attnT, in_=s_ps, func=AF.Exp)
                        return attnT

                    def av_block(attnT, jt):
                        first, last = (jt == 0), (jt == NJT - 1)
                        nc.tensor.matmul(av_e[0:65, :], v1e[:, jt, 0:65],
                                         attnT[:, 0:512],
                                         start=first, stop=last)
                        nc.tensor.matmul(av_o[:, :], v1o[:, jt, :],
                                         attnT[:, 512:1024],
                                         start=first, stop=last)

                    attnT_prev = exp_block(sim_block(0))
                    if debug_taps and pr == 0 and ih == 0:
                        nc.sync.dma_start(out=dbg['dbg_attnT'][:, :],
                                          in_=attnT_prev.bitcast(f32))
                    for jt in range(1, NJT):
                        s_ps = sim_block(jt)
                        av_block(attnT_prev, jt - 1)
                        attnT_prev = exp_block(s_ps)
                    av_block(attnT_prev, NJT - 1)
                    # normalize + scatter into aoT. reciprocal_approx_fast's
                    # custom-DVE uop only works at base partition 0, so: even
                    # side broadcasts the raw denominator (from PSUM row 64)
                    # then recips in place at base 0; odd side recips its
                    # base-0 PSUM row first, then broadcasts.
                    den = den_pool.tile([128, 512], f32, tag='den')
                    bce = bc_pool.tile([128, 512], f32, tag='bc')
                    bco = bc_pool.tile([128, 512], f32, tag='bc')
                    den_dr = dram_pool.tile([2, 512], f32, tag='dend')
                    nc.vector.tensor_copy(out=den[64:65, :], in_=av_e[64:65, :])
                    nc.sync.dma_start(out=den_dr[0:1, :], in_=den[64:65, :])
                    nc.sync.dma_start(out=bce[0:64, :],
                                      in_=den_dr[0:1, :].to_broadcast((64, 512)))
                    nc.vector.reciprocal_approx_fast(out=bce[0:64, :],
                                                     in_=bce[0:64, :])
                    nc.vector.reciprocal_approx_fast(out=den[0:1, :],
                                                     in_=av_o[0:1, :])
                    nc.sync.dma_start(out=den_dr[1:2, :], in_=den[0:1, :])
                    nc.sync.dma_start(out=bco[64:128, :],
                                      in_=den_dr[1:2, :].to_broadcast((64, 512)))
                    nc.vector.tensor_mul(
                        out=aoT_sb[0:64, pr, i0:i0 + 512],
                        in0=av_e[0:64, :], in1=bce[0:64, :])
                    nc.vector.tensor_mul(
                        out=aoT_sb[64:128, pr, i0:i0 + 512],
                        in0=av_o[64:128, :], in1=bco[64:128, :])
                    if debug_taps and pr == 0 and ih == 0:
                        av_e_sb = bc_pool.tile([128, 512], f32, tag='bc')
                        av_o_sb = bc_pool.tile([128, 512], f32, tag='bc')
                        nc.vector.tensor_copy(av_e_sb, av_e)
                        nc.vector.tensor_copy(av_o_sb, av_o)
                        nc.sync.dma_start(out=dbg['dbg_ave'][:, :], in_=av_e_sb)
                        nc.sync.dma_start(out=dbg['dbg_avo'][:, :], in_=av_o_sb)
                        nc.sync.dma_start(out=dbg['dbg_bce'][:, :], in_=bce)
                        nc.sync.dma_start(out=dbg['dbg_bco'][:, :], in_=bco)

            if debug_taps:
                nc.sync.dma_start(out=dbg['dbg_aoT'][:, :, :],
                                  in_=aoT_sb.bitcast(f32))

            # ---- phase D: out = aoT.T @ Wo ----
            for it in range(8):
                o_sb = out_pool.tile([128, D], f32, tag='osb')
                for mc, mw in ((0, 512), (512, 256)):
                    o_ps = ps_small.tile([128, 512], f32, tag='small')
                    for k in range(KD):
                        nc.tensor.matmul(
                            o_ps[:, 0:mw],
                            aoT_sb[:, k, it * 128:(it + 1) * 128],
                            wo_sb[:, k, mc:mc + mw],
                            start=(k == 0), stop=(k == KD - 1))
                    nc.vector.tensor_copy(o_sb[:, mc:mc + mw], o_ps[:, 0:mw])
                nc.sync.dma_start(out=out_d[it * 128:(it + 1) * 128, :], in_=o_sb)

    nc.compile()
    return nc


def _get_nc():
    if 'nc' not in _CACHE:
        _CACHE['nc'] = _build_nc()
    return _CACHE['nc']


def _numpy_fallback(x, context, mask, ln_w, Wq, Wkv, Wo):
    mu = x.mean(-1, keepdims=True)
    var = x.var(-1, keepdims=True)
    xn = (x - mu) / np.sqrt(var + EPS) * ln_w
    q = (xn @ Wq).reshape(B, N, HEADS, DH).transpose(0, 2, 1, 3) * SCALE
    kv = context @ Wkv
    k, v = kv[..., :DH], kv[..., DH:]
    sim = np.einsum('bhid,bjd->bhij', q, k) + mask[:, None, :, :]
    sim -= sim.max(-1, keepdims=True)
    a = np.exp(sim)
    a /= a.sum(-1, keepdims=True)
    out = np.einsum('bhij,bjd->bhid', a, v)
    return (out.transpose(0, 2, 1, 3).reshape(B, N, HEADS * DH) @ Wo).astype(
        np.float32)


def run_sharded(x, context, mask, ln_w, Wq, Wkv, Wo, trace=False):
    """Run the bass kernel on 8 cores; returns (out, BassKernelResults)."""
    from concourse.bass_utils import run_bass_kernel_spmd

    nc = _get_nc()
    lnw_s = np.ascontiguousarray(ln_w * SCALE, dtype=np.float32)
    in_maps = []
    for c in range(NCORES):
        b, half = divmod(c, 2)
        in_maps.append({
            'x': np.ascontiguousarray(x[b, half * NI:(half + 1) * NI, :]),
            'ctx': np.ascontiguousarray(context[b]),
            'lnw': lnw_s,
            'wq': np.ascontiguousarray(Wq),
            'wkv': np.ascontiguousarray(Wkv),
            'wo': np.ascontiguousarray(Wo),
        })
    res = run_bass_kernel_spmd(nc, in_maps, core_ids=list(range(NCORES)),
                               trace=trace)
    out = np.empty((B, N, D), dtype=np.float32)
    for c in range(NCORES):
        b, half = divmod(c, 2)
        out[b, half * NI:(half + 1) * NI, :] = res.results[c]['out']
    return out, res


def kernel(x, context, mask, ln_w, Wq, Wkv, Wo):
    x = np.asarray(x, dtype=np.float32)
    context = np.asarray(context, dtype=np.float32)
    mask = np.asarray(mask, dtype=np.float32)
    ln_w = np.asarray(ln_w, dtype=np.float32)
    Wq = np.asarray(Wq, dtype=np.float32)
    Wkv = np.asarray(Wkv, dtype=np.float32)
    Wo = np.asarray(Wo, dtype=np.float32)
    if mask.any():
        # The device kernel folds the (always-zero) additive mask away; a
        # nonzero mask is handled by the exact host fallback.
        return _numpy_fallback(x, context, mask, ln_w, Wq, Wkv, Wo)
    out, _ = run_sharded(x, context, mask, ln_w, Wq, Wkv, Wo)
    return out
